# revision 14
# baseline (speedup 1.0000x reference)
"""Trainium2 Bass kernel for ClaimValidationLoss.

Data-parallel over 8 NeuronCores: each core takes 32 of the 256 batches.
Instead of streaming the full 32MB adjacency shard through the core, the
kernel computes flat element offsets for its 32K claims on-device (DVE int
ops) and uses a GPSIMD indirect DMA to gather exactly the 32K probabilities
it needs straight from DRAM. The BCE transform runs on DVE/ACT, the per-core
(sum_log_q, n_valid) pair is reduced with a 1-column matmul, and the host
all-reduces the 8 pairs and does the final division.

Raw bacc (no TileContext): the program is a single linear dataflow, so
hand-placed semaphores avoid Tile's all-engine entry/exit barriers.
"""

import numpy as np

import concourse.bass as bass
from concourse import bacc, mybir
from concourse.bass_utils import run_bass_kernel_spmd

# Problem geometry (hardcoded per contest contract).
B, N, M = 256, 512, 1024
NCORES = 8
BL = B // NCORES            # 32 batches per core
P = 128                     # SBUF partitions
CF = BL * M // P            # 256 claims per partition
TOTAL = BL * N * N          # 8388608 adjacency elements per core
SHIFT_NN = 18               # log2(N*N)
SHIFT_BATCH = 2             # log2(P / BL): partition p holds batch p >> 2
EPS = float(np.float32(1e-7))
ONE_M_EPS = float(np.float32(1.0 - 1e-7))

f32 = mybir.dt.float32
i32 = mybir.dt.int32
Alu = mybir.AluOpType
Act = mybir.ActivationFunctionType

_CACHE = {}


def _build_nc():
    nc = bacc.Bacc("TRN2", target_bir_lowering=False, debug=False)

    adj = nc.dram_tensor("adj", [TOTAL, 1], f32, kind="ExternalInput")
    claims = nc.dram_tensor("claims", [P, 5 * CF], i32, kind="ExternalInput")
    out = nc.dram_tensor("out", [P, 2], f32, kind="ExternalOutput")

    cl = nc.alloc_sbuf_tensor("cl", [P, 5 * CF], i32)
    base = nc.alloc_sbuf_tensor("base", [P, 1], i32)
    off = nc.alloc_sbuf_tensor("off", [P, CF], i32)
    praw = nc.alloc_sbuf_tensor("praw", [P, CF], f32)
    s_t = nc.alloc_sbuf_tensor("s_t", [P, CF], f32)
    w_t = nc.alloc_sbuf_tensor("w_t", [P, CF], f32)
    q_t = nc.alloc_sbuf_tensor("q_t", [P, CF], f32)
    ai_t = nc.alloc_sbuf_tensor("ai_t", [P, CF], i32)
    is4_t = nc.alloc_sbuf_tensor("is4_t", [P, CF], i32)
    vf_t = nc.alloc_sbuf_tensor("vf_t", [P, CF], f32)
    lg_t = nc.alloc_sbuf_tensor("lg_t", [P, CF], f32)
    consts = nc.alloc_sbuf_tensor("consts", [P, 3], f32)   # [0.5, 1.0, 0.0]
    stats = nc.alloc_sbuf_tensor("stats", [P, 2], f32)     # [sum_log_q, n_valid]
    actwarm = nc.alloc_sbuf_tensor("actwarm", [P, 1], f32)

    s_ab = nc.alloc_semaphore("s_ab")       # claims va|vb DMA
    s_rest = nc.alloc_semaphore("s_rest")   # claims rt|tt|mk DMA
    s_base = nc.alloc_semaphore("s_base")   # base offsets ready
    s_off = nc.alloc_semaphore("s_off")     # gather offsets ready
    s_g = nc.alloc_semaphore("s_g")         # gather done
    s_vf = nc.alloc_semaphore("s_vf")       # n_valid column + consts ready
    s_q = nc.alloc_semaphore("s_q")         # q ready for Ln
    s_ln = nc.alloc_semaphore("s_ln")       # log column ready
    s_out = nc.alloc_semaphore("s_out")     # output DMA done

    va = cl.ap()[:, 0:CF]
    vb = cl.ap()[:, CF:2 * CF]
    rt = cl.ap()[:, 2 * CF:3 * CF]
    tt = cl.ap()[:, 3 * CF:4 * CF]
    mk = cl.ap()[:, 4 * CF:5 * CF]

    # ---- SYNC/SCALAR: input DMAs on two HWDGE queues in parallel ----
    nc.sync.dma_start(cl.ap()[:, 0:2 * CF], claims.ap()[:, 0:2 * CF]) \
        .then_inc(s_ab, 16)
    nc.scalar.dma_start(cl.ap()[:, 2 * CF:5 * CF], claims.ap()[:, 2 * CF:5 * CF]) \
        .then_inc(s_rest, 16)

    # ---- SCALAR: warm the Ln activation table while DMAs run ----
    nc.scalar.activation(out=actwarm.ap()[:, :], in_=actwarm.ap()[:, :],
                         func=Act.Ln, bias=1.0, scale=0.0)   # ln(0*x+1) = 0

    # ---- GPSIMD: base[p] = p (shifted into batch*N*N on DVE below) ----
    nc.gpsimd.iota(base.ap()[:, :], pattern=[[0, 1]], base=0, channel_multiplier=1)
    nc.gpsimd.maybe_drain_then_inc((s_base, 1))

    # ---- VECTOR: constants (no deps) ----
    nc.vector.memset(consts.ap()[:, 0:1], 0.5)
    nc.vector.memset(consts.ap()[:, 1:2], 1.0)
    nc.vector.memset(consts.ap()[:, 2:3], 0.0)

    # ---- VECTOR: offsets once va|vb there ----
    nc.vector.wait_ge(s_base, 1)
    nc.vector.tensor_scalar(out=base.ap()[:, :], in0=base.ap()[:, :],
                            scalar1=SHIFT_BATCH, scalar2=SHIFT_NN,
                            op0=Alu.arith_shift_right, op1=Alu.logical_shift_left)
    nc.vector.wait_ge(s_ab, 16)
    nc.vector.scalar_tensor_tensor(out=off.ap()[:, :], in0=va, scalar=N, in1=vb,
                                   op0=Alu.mult, op1=Alu.add)
    nc.vector.drain()
    nc.vector.tensor_tensor(out=off.ap()[:, :], in0=off.ap()[:, :],
                            in1=base.ap()[:, 0:1].to_broadcast([P, CF]),
                            op=Alu.add)
    nc.vector.maybe_drain_then_inc((s_off, 1))

    # ---- GPSIMD: the gather ----
    nc.gpsimd.wait_ge(s_off, 1)
    nc.gpsimd.indirect_dma_start(
        out=praw.ap()[:, :], out_offset=None, in_=adj.ap()[:, :],
        in_offset=bass.IndirectOffsetOnAxis(ap=off.ap()[:, :], axis=0)) \
        .then_inc(s_g, 16)

    # ---- VECTOR: coefficient prep under the gather.
    # q = praw*w + s reproduces every case:
    #   normal claims:  s = (rt&1 == is_true), w = 1-2s  -> q = p or 1-p
    #   rt >= 4:        s = 0.5, w = 0                   -> q = 0.5
    #   padded:         s = 1,   w = 0                   -> q = 1, ln q ~ 0
    nc.vector.wait_ge(s_rest, 16)
    nc.vector.tensor_scalar(out=ai_t.ap()[:, :], in0=rt, scalar1=1, scalar2=None,
                            op0=Alu.bitwise_and)
    nc.vector.tensor_scalar(out=is4_t.ap()[:, :], in0=rt, scalar1=4, scalar2=None,
                            op0=Alu.is_ge)
    nc.vector.tensor_scalar(out=vf_t.ap()[:, :], in0=mk, scalar1=0, scalar2=None,
                            op0=Alu.is_equal)
    nc.vector.drain()
    nc.vector.tensor_tensor(out=s_t.ap()[:, :], in0=ai_t.ap()[:, :], in1=tt,
                            op=Alu.is_equal)
    nc.vector.tensor_reduce(out=stats.ap()[:, 1:2], in_=vf_t.ap()[:, :],
                            axis=mybir.AxisListType.X, op=Alu.add)
    nc.vector.drain()
    nc.vector.copy_predicated(out=s_t.ap()[:, :], mask=is4_t.ap()[:, :],
                              data=consts.ap()[:, 0:1].to_broadcast([P, CF]))
    nc.vector.drain()
    nc.vector.tensor_scalar(out=w_t.ap()[:, :], in0=s_t.ap()[:, :],
                            scalar1=-2.0, scalar2=1.0,
                            op0=Alu.mult, op1=Alu.add)
    nc.vector.drain()
    nc.vector.copy_predicated(out=w_t.ap()[:, :], mask=mk,
                              data=consts.ap()[:, 2:3].to_broadcast([P, CF]))
    nc.vector.copy_predicated(out=s_t.ap()[:, :], mask=mk,
                              data=consts.ap()[:, 1:2].to_broadcast([P, CF]))
    nc.vector.maybe_drain_then_inc((s_vf, 1))

    # ---- VECTOR: q once the gather lands (3-op critical chain) ----
    nc.vector.wait_ge(s_g, 16)
    nc.vector.tensor_tensor(out=q_t.ap()[:, :], in0=praw.ap()[:, :],
                            in1=w_t.ap()[:, :], op=Alu.mult)
    nc.vector.drain()
    nc.vector.tensor_tensor(out=q_t.ap()[:, :], in0=q_t.ap()[:, :],
                            in1=s_t.ap()[:, :], op=Alu.add)
    nc.vector.drain()
    nc.vector.tensor_scalar(out=q_t.ap()[:, :], in0=q_t.ap()[:, :],
                            scalar1=EPS, scalar2=ONE_M_EPS,
                            op0=Alu.max, op1=Alu.min)
    nc.vector.maybe_drain_then_inc((s_q, 1))

    # ---- SCALAR: lg = ln(q), stats[:,0] = sum_k lg ----
    nc.scalar.wait_ge(s_q, 1)
    nc.scalar.activation(out=lg_t.ap()[:, :], in_=q_t.ap()[:, :], func=Act.Ln,
                         accum_out=stats.ap()[:, 0:1])
    nc.scalar.maybe_drain_then_inc((s_ln, 1))

    # ---- SYNC: ship per-partition stats; host does the tiny all-reduce ----
    nc.sync.wait_ge(s_ln, 1)
    nc.sync.wait_ge(s_vf, 1)
    nc.sync.dma_start(out.ap()[:, :], stats.ap()[:, :]).then_inc(s_out, 16)
    nc.sync.wait_ge(s_out, 16)

    nc.compile()
    return nc


def kernel(posterior_adjacency, var_a, var_b, relation_type, is_true, claim_mask):
    adj = np.asarray(posterior_adjacency, dtype=np.float32)
    va = np.asarray(var_a, dtype=np.int32)
    vb = np.asarray(var_b, dtype=np.int32)
    rt = np.asarray(relation_type, dtype=np.int32)
    tt = np.asarray(is_true, dtype=np.int32)
    mk = np.asarray(claim_mask).astype(np.int32)

    if "nc" not in _CACHE:
        _CACHE["nc"] = _build_nc()
    nc = _CACHE["nc"]

    in_maps = []
    for c in range(NCORES):
        sl = slice(c * BL, (c + 1) * BL)
        in_maps.append({
            "adj": np.ascontiguousarray(adj[sl]).reshape(TOTAL, 1),
            "claims": np.concatenate(
                [va[sl].reshape(P, CF), vb[sl].reshape(P, CF),
                 rt[sl].reshape(P, CF), tt[sl].reshape(P, CF),
                 mk[sl].reshape(P, CF)], axis=1),
        })

    res = run_bass_kernel_spmd(nc, in_maps, core_ids=list(range(NCORES)))
    pairs = np.stack([r["out"] for r in res.results]).astype(np.float64)
    sum_log_q = pairs[:, :, 0].sum()
    n_valid = pairs[:, :, 1].sum()
    if n_valid > 0:
        loss = -sum_log_q / max(n_valid, 1.0)
    else:
        loss = 0.0
    return np.float32(loss)
